# revision 16
# baseline (speedup 1.0000x reference)
"""Trainium2 Bass kernel for nn_Choquet_Integral.

Reformulation: the Choquet integral (sort + successive diffs + FM lattice
gather + einsum) equals a Mobius-transform contraction over subset minima:

    y[b, h] = sum_{T subset of {0..7}, T nonempty} mHat[T, h] * min_{i in T} x_b[i]

where mHat is the Mobius transform of the fuzzy measure FM (host-computed,
255 x 8). Subset minima come from min(a,b) = a - relu(a-b) in a 3-level
balanced cascade, so everything is constant-matrix matmuls (PE) interleaved
with relu drains (ACT/DVE). No sort, no gather.

This version is organized around the TRN2 cost model, where a matmul costs
out_free_size x pe_cycle regardless of K/M. Per 16-sample block:
  1x S1   (x16 [128,512] -> all 64 d2 diffs)
  4x S2   (z4 group tiles [0:48] -> 72 d4 diffs per group)
  32x S3  (900 d8 diffs per 4-sample group, 8 chunks of <=128 rows)
  32x y   (255-feature contraction per sample, 16-sample slotted PSUM)
  1x LN-stats
= 70 matmuls/block (vs 96 for a pair-based schedule).

Partition-alignment rules (walrus verifier): engine ops need 32-aligned
partition bases; f32r matmul PSUM outs at base 0; GPSIMD (Pool) cannot touch
PSUM. So every engine drain lands at base 0 of a staging tile and DMA (which
is offset-free) scatters into the Z tiles.

Sharding: data-parallel over N across the 8 NeuronCores (256 samples each).
"""

import sys

for _p in ("/opt/trn_rl_repo", "/root/.axon_site/_ro/trn_rl_repo"):
    if _p not in sys.path:
        sys.path.append(_p)

import numpy as np

import concourse.bass as bass
import concourse.bacc as bacc
import concourse.tile as tile
from concourse import mybir
from concourse.bass_utils import run_bass_kernel_spmd

N, S, D, H = 2048, 8, 512, 8
NCORES = 8
NPC = N // NCORES  # samples per core
NBLK = NPC // 16   # 16-sample blocks per core
LN_EPS = 1e-5
F32 = mybir.dt.float32
F32R = mybir.dt.float32r

NZ = 255   # features per sample: 8 x | 4 d2 | 18 d4 | 225 d8
NF8 = 900  # d8 features per 4-sample group

# chunk feature ranges within the 900 group-features (f = 225*s + j)
CHUNK_F = [range(128 * c, 128 * c + 128) for c in range(6)] + [
    range(768, 892),
    range(892, 900),
]

# Z tile row layout (per 4-sample group, 128 partitions):
#   0:32   x      (sample s rows 8s:8s+8)      [DMA from DRAM]
#   32:48  d2     (sample s rows 32+4s)        [DMA from d2stage]
#   48:120 d4     (sample s rows 48+18s)       [DMA from d4stage]
#   120:128 d8x   (chunk-7 relu'd d8 rows)     [DMA from its w tile]


def _zrow(s, r):
    """z-tile partition row of per-sample feature r (baseline Z-coords 0..29)."""
    if r < 8:
        return 8 * s + r
    if r < 12:
        return 32 + 4 * s + (r - 8)
    return 48 + 18 * s + (r - 12)


# --------------------------------------------------------------------------
# Host-side constant matrices
# --------------------------------------------------------------------------
def _build_structure():
    """FM-independent pieces: A1 [8,4], A2 [12,18], A3 [30,225], and the
    linear forms of every subset minimum over the 255-dim Z vector."""

    def v_x(i):
        v = np.zeros(NZ)
        v[i] = 1.0
        return v

    def e(row):
        v = np.zeros(NZ)
        v[row] = 1.0
        return v

    # relu convention: min(a, b) = a - relu(a - b); row e(.) holds relu(diff)
    m2 = [v_x(2 * p) - e(8 + p) for p in range(4)]

    def P(p, a):  # pair p value for local mask a in {1,2,3}
        return (v_x(2 * p), v_x(2 * p + 1), m2[p])[a - 1]

    m4 = {0: {}, 1: {}}
    d4rows = {0: {}, 1: {}}
    for side in range(2):
        p0, p1 = (0, 1) if side == 0 else (2, 3)
        for t in range(1, 16):
            a, b = t & 3, t >> 2
            if b == 0:
                m4[side][t] = P(p0, a)
            elif a == 0:
                m4[side][t] = P(p1, b)
            else:
                d4rows[side][(a, b)] = P(p0, a) - P(p1, b)
                m4[side][t] = P(p0, a) - e(12 + 9 * side + 3 * (a - 1) + (b - 1))

    d8rows = {}
    minT = {}
    for T in range(1, 256):
        t, u = T & 15, T >> 4
        if u == 0:
            minT[T] = m4[0][t]
        elif t == 0:
            minT[T] = m4[1][u]
        else:
            d8rows[(t, u)] = m4[0][t] - m4[1][u]
            minT[T] = m4[0][t] - e(30 + 15 * (t - 1) + (u - 1))

    A1 = np.zeros((8, 4))
    for p in range(4):
        A1[2 * p, p] = 1.0
        A1[2 * p + 1, p] = -1.0

    A2 = np.zeros((12, 18))
    for side in range(2):
        for a in range(1, 4):
            for b in range(1, 4):
                A2[:, 9 * side + 3 * (a - 1) + (b - 1)] = d4rows[side][(a, b)][:12]

    A3 = np.zeros((30, 225))
    for t in range(1, 16):
        for u in range(1, 16):
            A3[:, 15 * (t - 1) + (u - 1)] = d8rows[(t, u)][:30]

    return A1, A2, A3, minT


_A1, _A2, _A3, _MINT = _build_structure()


def _mobius(FM):
    """mHat[T, h], T in [0, 255]; mu(mask) = FM[mask-1], mu(0) = 0."""
    mh = np.zeros((256, H), np.float64)
    mh[1:] = FM.astype(np.float64)
    for b in range(8):
        bit = 1 << b
        idx = np.arange(256)
        hi = idx[(idx & bit) != 0]
        mh[hi] -= mh[hi ^ bit]
    return mh


def _host_matrices(FM):
    """All lhsT constants for the block schedule."""
    mh = _mobius(FM)
    C = np.zeros((NZ, H))
    for T in range(1, 256):
        C += np.outer(_MINT[T], mh[T])
    f = np.float32

    # S1: x16 [128,512] (sample k rows 8k) -> d2p rows 4k+p
    A1blk = np.zeros((128, 128), f)
    for k in range(16):
        for p in range(4):
            A1blk[8 * k + 2 * p, 4 * k + p] = 1.0
            A1blk[8 * k + 2 * p + 1, 4 * k + p] = -1.0

    # S2: z[0:48] (x+d2 of 4 samples) -> 72 d4 diffs
    A2blk = np.zeros((48, 72), f)
    for s in range(4):
        A2blk[8 * s : 8 * s + 8, 18 * s : 18 * s + 18] = _A2[0:8]
        A2blk[32 + 4 * s : 36 + 4 * s, 18 * s : 18 * s + 18] = _A2[8:12]

    # G [120, 900]: group feature f = 225*s + j as linear form over z rows
    G = np.zeros((120, 900))
    for s in range(4):
        for r in range(30):
            G[_zrow(s, r), 225 * s : 225 * s + 225] = _A3[r]

    # S3 chunk lhsTs [120, 128] (zero-padded output columns)
    CH = []
    for c in range(8):
        m = np.zeros((120, 128), f)
        feats = list(CHUNK_F[c])
        m[:, 0 : len(feats)] = G[:, feats]
        CH.append(m)

    # y lhsTs. YZ[g] contracts z[0:128] of group g; YW[c][g] contracts w
    # chunk c. Output columns are the 16-sample slots 8k..8k+8 of y16.
    YZ = []
    for g in range(4):
        m = np.zeros((128, 128), f)
        for s in range(4):
            k = 4 * g + s
            for r in range(30):
                m[_zrow(s, r), 8 * k : 8 * k + 8] = C[r]
        for i, fidx in enumerate(CHUNK_F[7]):  # d8x rows 120:128 (sample 3)
            s, j = divmod(fidx, 225)
            m[120 + i, 8 * (4 * g + s) : 8 * (4 * g + s) + 8] = C[30 + j]
        YZ.append(m)

    YW = []
    for c in range(7):
        per_g = []
        for g in range(4):
            m = np.zeros((128, 128), f)
            for i, fidx in enumerate(CHUNK_F[c]):
                s, j = divmod(fidx, 225)
                m[i, 8 * (4 * g + s) : 8 * (4 * g + s) + 8] = C[30 + j]
            per_g.append(m)
        YW.append(per_g)

    return {"a1": A1blk, "a2": A2blk.astype(f), "ch": CH, "yz": YZ, "yw": YW}


def _g16():
    """Block-diagonal ones [128, 128]: per-sample (8-row group) sum replicator."""
    g = np.zeros((128, 128), np.float32)
    for k in range(16):
        g[8 * k : 8 * k + 8, 8 * k : 8 * k + 8] = 1.0
    return g


# cpack column layout: every constant packed into one [128, CP] f32 tensor so
# the whole preamble is a single DMA.
def _cp_layout():
    """Three segments so the preamble DMA can be split: segment 1 (a1/a2/ch)
    unblocks the whole front+S3 path, segment 2 (yz/yw/g16) is needed one
    group later, segment 3 (LN consts) a full block later."""
    cols = {}
    c = 0
    marks = []

    def put(name, w, rows):
        nonlocal c
        cols[name] = (c, w, rows)
        c += w

    put("a1", 128, 128)
    put("a2", 72, 48)
    for i in range(8):
        put(f"ch{i}", 128, 120)
    marks.append(c)
    for g in range(4):
        put(f"yz{g}", 128, 128)
    for ci in range(7):
        for g in range(4):
            put(f"yw{ci}_{g}", 128, 128)
    put("g16", 128, 128)
    marks.append(c)
    put("lnw", D, 128)
    put("lnb", D, 128)
    put("pre", 1, 128)
    put("eps", 1, 128)
    return cols, c, marks


_CPCOLS, CP, _CP_MARKS = _cp_layout()


def _pack_consts(mats, lnw, lnb, pre_w):
    cp = np.zeros((128, CP), np.float32)

    def put(name, arr):
        c0, w, rows = _CPCOLS[name]
        cp[: arr.shape[0], c0 : c0 + arr.shape[1]] = arr

    put("a1", mats["a1"])
    put("a2", mats["a2"])
    for i in range(8):
        put(f"ch{i}", mats["ch"][i])
    for g in range(4):
        put(f"yz{g}", mats["yz"][g])
    for ci in range(7):
        for g in range(4):
            put(f"yw{ci}_{g}", mats["yw"][ci][g])
    put("g16", _g16())
    put("lnw", lnw)
    put("lnb", lnb)
    cp[:, _CPCOLS["pre"][0]] = pre_w
    cp[:, _CPCOLS["eps"][0]] = LN_EPS
    return cp


# --------------------------------------------------------------------------
# Bass module
# --------------------------------------------------------------------------
def build_module(npc=NPC, mm_dtype=F32R):
    nblk = npc // 16
    nc = bacc.Bacc("TRN2", target_bir_lowering=False, debug=False)

    x_in = nc.dram_tensor("x", [npc, S, D], mm_dtype, kind="ExternalInput").ap()
    y_out = nc.dram_tensor("y", [npc, H, D], F32, kind="ExternalOutput").ap()
    cpack = nc.dram_tensor("cpack", [128, CP], mm_dtype, kind="ExternalInput").ap()

    AluOp = mybir.AluOpType
    Act = mybir.ActivationFunctionType

    # ---- persistent SBUF constants ----
    cpk = nc.alloc_sbuf_tensor("cpk", [128, CP], mm_dtype).ap()

    def cslice(name, bitcast=None):
        c0, w, rows = _CPCOLS[name]
        ap = cpk[0:rows, c0 : c0 + w]
        return ap.bitcast(bitcast) if bitcast is not None else ap

    ct = {k: cslice(k) for k in _CPCOLS if k not in ("g16", "lnw", "lnb", "pre", "eps")}
    ct["g16"] = cslice("g16", bitcast=F32)
    lnw = cslice("lnw", bitcast=F32)
    lnb = cslice("lnb", bitcast=F32)
    pre = cslice("pre", bitcast=F32)
    eps = cslice("eps", bitcast=F32)

    # ---- SBUF working tensors (double-buffered by block parity) ----
    X16 = [nc.alloc_sbuf_tensor(f"x16_{i}", [128, D], mm_dtype).ap() for i in range(2)]
    Z = [nc.alloc_sbuf_tensor(f"z_{i}", [128, 4 * D], mm_dtype).ap() for i in range(2)]
    D2S = [nc.alloc_sbuf_tensor(f"d2s_{i}", [64, D], mm_dtype).ap() for i in range(2)]
    D4S = [nc.alloc_sbuf_tensor(f"d4s_{i}", [72, 4 * D], mm_dtype).ap() for i in range(2)]

    # ---- PSUM: 2 y16 banks + 3 pair tensors (6 half-slots) ----
    Y16 = [nc.alloc_psum_tensor(f"y16_{i}", [128, D], F32).ap() for i in range(2)]
    PP = [nc.alloc_psum_tensor(f"pp{i}", [128, 2 * D], F32).ap() for i in range(3)]

    def mm(out, lhsT, rhs, **kw):
        nc.tensor.matmul(out, lhsT, rhs, **kw)

    E1, E2 = _CP_MARKS
    with tile.TileContext(nc) as tc0:
        nc.sync.dma_start(out=cpk[:, 0:E1], in_=cpack[:, 0:E1])

    with tile.TileContext(nc) as tc:
        with (
            tc.tile_pool(name="wpool", bufs=8) as wpool,
            tc.tile_pool(name="lnpool", bufs=2) as lnpool,
        ):
            # ---------- psum half-slot rotation ----------
            rot = {"i": 0}
            pending = {"v": None}  # (tensor_idx, chunk_key) for an undrained even half
            wmap = {}   # (b, g, c) -> (w_tile_ap, half)
            drain_tog = {"i": 0}

            def drain_engine():
                drain_tog["i"] ^= 1
                return nc.scalar if drain_tog["i"] else nc.vector

            def do_drain(src_ap, dst_ap, eng):
                if eng is nc.scalar:
                    nc.scalar.activation(out=dst_ap, in_=src_ap, func=Act.Relu)
                else:
                    nc.vector.tensor_scalar(
                        out=dst_ap, in0=src_ap, scalar1=0.0, scalar2=None,
                        op0=AluOp.max,
                    )

            def flush_pending():
                """Solo-drain an undrained even-half chunk."""
                if pending["v"] is None:
                    return
                ti, key, rows = pending["v"]
                pending["v"] = None
                w = wpool.tile([128, 2 * D], mm_dtype, tag="w")
                eng = drain_engine()
                do_drain(PP[ti][0:rows, 0:D], w[0:rows, 0:D], eng)
                wmap[key] = (w, 0)

            def alloc_half(kind, key=None, rows=128):
                """Advance rotation; returns (tensor_idx, half). kind is
                'chunk' or 'insert'. Chunk halves get drained (paired when
                both halves of a tensor hold back-to-back chunks)."""
                i = rot["i"]
                rot["i"] = (i + 1) % 6
                ti, half = divmod(i, 2)
                if half == 0:
                    flush_pending()
                return ti, half

            def s3_mm(b, g, c):
                z = Z[b % 2]
                ti, half = alloc_half("chunk")
                out = PP[ti][0:128, half * D : half * D + D]
                mm(out, ct[f"ch{c}"], z[0:120, g * D : g * D + D])
                key = (b, g, c)
                if half == 0:
                    pending["v"] = (ti, key, 128)
                else:
                    if pending["v"] is not None and pending["v"][0] == ti:
                        # pair drain: both halves in one op
                        _, key0, _ = pending["v"]
                        pending["v"] = None
                        w = wpool.tile([128, 2 * D], mm_dtype, tag="w")
                        eng = drain_engine()
                        do_drain(PP[ti][0:128, 0 : 2 * D], w[0:128, 0 : 2 * D], eng)
                        wmap[key0] = (w, 0)
                        wmap[key] = (w, 1)
                    else:
                        flush_pending()
                        w = wpool.tile([128, 2 * D], mm_dtype, tag="w")
                        eng = drain_engine()
                        do_drain(PP[ti][0:128, D : 2 * D], w[0:128, 0:D], eng)
                        wmap[key] = (w, 0)
                if c == 7:
                    # route the 8 ragged relu'd d8 rows into z[120:128, group]
                    if key not in wmap:
                        flush_pending()  # c7 landed on an even half
                    w, wh = wmap[key]
                    nc.sync.dma_start(
                        out=z[120:128, g * D : g * D + D],
                        in_=w[0:8, wh * D : wh * D + D],
                    )

            def insert_slot():
                """Claim a half-slot for d2p/d4p/stats; returns psum AP
                [128, D] at base 0 of that half."""
                ti, half = alloc_half("insert")
                if half == 1:
                    flush_pending()
                return PP[ti][0:128, half * D : half * D + D]

            def y_mm(b, g, c, first, last):
                yb = Y16[b % 2]
                if c < 7:
                    w, half = wmap.pop((b, g, c))
                    mm(
                        yb[:, :], ct[f"yw{c}_{g}"],
                        w[0:128, half * D : half * D + D],
                        start=first, stop=last,
                    )
                else:
                    wmap.pop((b, g, 7), None)
                    z = Z[b % 2]
                    mm(
                        yb[:, :], ct[f"yz{g}"], z[0:128, g * D : g * D + D],
                        start=first, stop=last,
                    )

            # ---------- front (per-block input pipeline) ----------
            def front_dma(b):
                # x16: 16 samples x 8 rows
                nc.sync.dma_start(
                    out=X16[b % 2],
                    in_=x_in[b * 16 : b * 16 + 16].rearrange("n s d -> (n s) d"),
                )
                # z x-rows: sample (4g+s) row r -> partition 8s+r, col-group g
                nc.sync.dma_start(
                    out=Z[b % 2][0:32, :].rearrange("p (g d) -> p g d", g=4),
                    in_=x_in[b * 16 : b * 16 + 16].rearrange(
                        "(g s) r d -> (s r) g d", g=4
                    ),
                )

            def s1(b):
                d2p = insert_slot()
                mm(d2p[0:128, :], ct["a1"], X16[b % 2][:, :])
                eng = drain_engine()
                do_drain(d2p[0:64, :], D2S[b % 2][0:64, :], eng)
                # scatter: d2 rows 4k+p -> z[32 + 4s + p, col-group g]
                for g in range(4):
                    nc.sync.dma_start(
                        out=Z[b % 2][32:48, g * D : g * D + D],
                        in_=D2S[b % 2][16 * g : 16 * g + 16, :],
                    )

            def s2(b, g):
                d4p = insert_slot()
                z = Z[b % 2]
                mm(d4p[0:72, :], ct["a2"], z[0:48, g * D : g * D + D])
                eng = drain_engine()
                do_drain(d4p[0:72, :], D4S[b % 2][0:72, g * D : g * D + D], eng)
                if g in (1, 3):  # one DMA per pair of groups
                    c0 = (g - 1) * D
                    nc.sync.dma_start(
                        out=z[48:120, c0 : c0 + 2 * D],
                        in_=D4S[b % 2][0:72, c0 : c0 + 2 * D],
                    )

            # ---------- LayerNorm ----------
            ln_mid = {}

            def ln_stats(b):
                yb = Y16[b % 2]
                r = lnpool.tile([128, 2], F32, tag="r")
                sq = lnpool.tile([128, D], F32, tag="sq")
                nc.vector.tensor_reduce(
                    out=r[:, 0:1], in_=yb[:, :], axis=mybir.AxisListType.X,
                    op=AluOp.add,
                )
                nc.scalar.activation(
                    out=sq[:], in_=yb[:, :], func=Act.Square, accum_out=r[:, 1:2]
                )
                sp = insert_slot()
                mm(sp[0:128, 0:2], ct["g16"], r[:, :])
                mean = lnpool.tile([128, 1], F32, tag="mean")
                var = lnpool.tile([128, 1], F32, tag="var")
                rstd = lnpool.tile([128, 1], F32, tag="rstd")
                m2t = lnpool.tile([128, 1], F32, tag="m2t")
                nc.vector.tensor_scalar(
                    out=mean[:], in0=sp[0:128, 0:1], scalar1=1.0 / (H * D),
                    scalar2=None, op0=AluOp.mult,
                )
                nc.gpsimd.tensor_tensor(
                    out=m2t[:], in0=mean[:], in1=mean[:], op=AluOp.mult
                )
                nc.vector.scalar_tensor_tensor(
                    out=var[:], in0=sp[0:128, 1:2], scalar=1.0 / (H * D),
                    in1=m2t[:], op0=AluOp.mult, op1=AluOp.subtract,
                )
                nc.scalar.activation(out=rstd[:], in_=var[:], func=Act.Sqrt, bias=eps)
                nc.vector.reciprocal(out=rstd[:], in_=rstd[:])
                ln_mid[b] = (mean, rstd)

            def ln_apply(b):
                yb = Y16[b % 2]
                mean, rstd = ln_mid.pop(b)
                t1 = lnpool.tile([128, D], F32, tag="t1")
                nc.vector.tensor_scalar(
                    out=t1[:], in0=yb[:, :], scalar1=mean[:], scalar2=rstd[:],
                    op0=AluOp.subtract, op1=AluOp.mult,
                )
                t2 = lnpool.tile([128, D], F32, tag="t2")
                nc.gpsimd.tensor_tensor(out=t2[:], in0=t1[:], in1=lnw, op=AluOp.mult)
                nc.gpsimd.tensor_tensor(out=t2[:], in0=t2[:], in1=lnb, op=AluOp.add)
                u = lnpool.tile([128, D], F32, tag="u")
                v = lnpool.tile([128, D], F32, tag="v")
                o16 = lnpool.tile([128, D], F32, tag="o16")
                nc.gpsimd.tensor_scalar(
                    out=u[:], in0=t2[:], scalar1=0.0, scalar2=pre,
                    op0=AluOp.min, op1=AluOp.mult,
                )
                nc.gpsimd.tensor_scalar(
                    out=v[:], in0=t2[:], scalar1=0.0, scalar2=None, op0=AluOp.max,
                )
                nc.gpsimd.tensor_tensor(out=o16[:], in0=u[:], in1=v[:], op=AluOp.add)
                nc.sync.dma_start(
                    out=y_out[b * 16 : b * 16 + 16].rearrange("n h d -> (n h) d"),
                    in_=o16[:],
                )

            # ---------- main schedule ----------
            # rest of the constants arrive while the front(0) chain runs
            nc.sync.dma_start(out=cpk[:, E1:E2], in_=cpack[:, E1:E2])
            nc.sync.dma_start(out=cpk[:, E2:CP], in_=cpack[:, E2:CP])
            # front(0) preamble
            front_dma(0)
            s1(0)
            for g in range(4):
                s2(0, g)

            prev = None  # (b, g) whose y matmuls are pending

            def emit_y(c):
                if prev is None:
                    return
                b, g = prev
                first = (g == 0 and c == 0)
                last = (g == 3 and c == 7)
                y_mm(b, g, c, first, last)

            for b in range(nblk):
                for g in range(4):
                    for c in range(8):
                        s3_mm(b, g, c)
                        # Front work for block b+1.  Emission order matters:
                        # Z/X16/D2S/D4S writers must be emitted AFTER the
                        # final readers of the previous block on the same
                        # parity — i.e. after (b-1, g3)'s y matmuls, which
                        # are emitted during (b, g0).
                        if b + 1 < nblk:
                            if g == 1 and c == 0:
                                front_dma(b + 1)
                            elif g == 1 and c == 4:
                                s1(b + 1)
                            elif g == 2 and c == 4:
                                s2(b + 1, 0)
                            elif g == 3 and c == 0:
                                s2(b + 1, 1)
                            elif g == 3 and c == 2:
                                s2(b + 1, 2)
                            elif g == 3 and c == 4:
                                s2(b + 1, 3)
                        if b >= 1:
                            if g == 2 and c == 0:
                                ln_stats(b - 1)
                            elif g == 3 and c == 6:
                                ln_apply(b - 1)
                        emit_y(c)
                    prev = (b, g)
            # drain the pipeline: y of the final group, LN of last blocks
            flush_pending()
            for c in range(8):
                emit_y(c)
            ln_stats(nblk - 1)
            ln_apply(nblk - 1)

    nc.compile()
    return nc


# --------------------------------------------------------------------------
# Entry point
# --------------------------------------------------------------------------
_CACHED = {}


def _get_module():
    if "nc" not in _CACHED:
        _CACHED["nc"] = build_module()
    return _CACHED["nc"]


def host_feeds(FM, ln_weight, ln_bias, prelu_w):
    mats = _host_matrices(np.asarray(FM, np.float64))
    lnw = np.tile(
        np.asarray(ln_weight, np.float32).reshape(1, H, D), (16, 1, 1)
    ).reshape(128, D)
    lnb = np.tile(
        np.asarray(ln_bias, np.float32).reshape(1, H, D), (16, 1, 1)
    ).reshape(128, D)
    return _pack_consts(mats, lnw, lnb, float(np.asarray(prelu_w).reshape(-1)[0]))


def kernel(x, FM, ln_weight, ln_bias, prelu_w):
    x = np.ascontiguousarray(np.asarray(x, np.float32))
    cpack = np.ascontiguousarray(host_feeds(FM, ln_weight, ln_bias, prelu_w))

    nc = _get_module()
    in_maps = []
    for c in range(NCORES):
        in_maps.append(
            {"x": np.ascontiguousarray(x[c * NPC : (c + 1) * NPC]), "cpack": cpack}
        )

    res = run_bass_kernel_spmd(nc, in_maps, core_ids=list(range(NCORES)))
    out = np.concatenate([r["y"] for r in res.results], axis=0)
    return out.astype(np.float32)


if __name__ == "__main__":
    nc = build_module(npc=32)
    print("module built ok")
